# revision 1
# baseline (speedup 1.0000x reference)
"""Bass/Trainium2 kernel for nn_F_Loss_65446711656630.

Strategy (data-parallel over N, 8 cores):
  - Host: GLOBAL stable sort of all rows by class id, quantize to fp8 e4m3
    (final loss rel err ~1.4e-4, well under tolerance; halves HBM traffic
    vs fp16 to ~4.2 MiB/core), then lay out per-core operands: blocks 0-2
    row-major for the PE ([granule, partition, half, chunk, 3x(128 feat +
    ones col)]), block 3 feature-major for ACT/DVE.
  - Device (static kernel), work split across engines:
      * TensorE (blocks 0-2): for each 128-row chunk, one fp8 matmul with
        stationary = X_b and moving = [X_b | 1] accumulates X_b^T X_b
        (diag = per-feature sum of squares) and X_b^T 1 (per-feature sums)
        into PSUM. (fp8 DoubleRow loses here: its dual weight load costs
        256 LDWEIGHTS columns per 129-column matmul.) 16 chunks accumulate
        per 2048-row granule; granule stats stage through SBUF via one ACT
        copy. The stream runs at ~71ns/matmul, LDWEIGHTS-paced.
      * ACT (block 3): Square activation with accum_out -> half-granule
        sum of squares.
      * DVE (block 3): TensorReduce -> per-chunk sums.
    PE ~13.7us, ACT ~9us, DVE ~10us, overlapping ~13us of DMA; DMA trigger
    count is minimized (each costs ~610ns of serial SP sequencer time).
  - Host: per-class stats from single-class granule partials (f64) + direct
    numpy f64 sums for the few class-transition granules; then the tiny
    O(C^2 D) pairwise betainc/top-k stage in f32 jax on CPU (mirroring the
    reference's numerics exactly).
"""

import os

import ml_dtypes
import numpy as np

# safety net: recover cleanly if a previous process left a NeuronCore wedged
os.environ.setdefault("NEURON_RT_RESET_CORES", "1")

C = 16
D = 512
N = 65536
NCORES = 8
ROWS = N // NCORES          # 8192 rows per core
P = 128                     # SBUF partitions
NBLK = 4                    # feature blocks of 128
PEBLK = 3                   # blocks computed on the TensorEngine
BCOL = P + 1                # 129 columns per block: 128 features + ones col
GRAN = 2048                 # rows per granule (stats accumulation unit)
NGRAN = ROWS // GRAN        # 4 granules per core
NCHK = GRAN // P            # 16 chunks per granule
HCHK = NCHK // 2            # 8 chunks per half-granule DMA
NHALF = NGRAN * 2
XMIN, XMAX = 1e-37, 1.0 - 1e-5

F8 = ml_dtypes.float8_e4m3

_NC_CACHE = {}


def _build_nc():
    """Per-core SPMD program.

    Inputs:  "hta"   [4, 128, 2, 8, 387] fp8e4, row-major for the PE
             (granule, partition, half, chunk, 3 blocks x [128 feat | 1.0];
              row r within granule = (half*8 + chunk)*128 + p)
             "htb"   [4, 128, 2, 1024] fp8e4, feature-major for ACT/DVE
             (granule, feature 384+p, half, row within half-granule)
    Outputs: "stats" [4, 128, 3, 129] f32   (PE blocks 0-2: stats[g,f,b,c] =
               sum over granule g of X[:,b*128+f]*X[:,b*128+c] for c<128,
               plain sum of X[:,b*128+f] at c==128)
             "blk3"  [128, 72] f32  (cols 0..7: ACT half-granule sumsq of
               feature 384+p, granule g at cols 2g/2g+1; cols 8..71: DVE
               chunk sums, granule g at cols 8+16g..8+16g+15)
    """
    import concourse.tile as tile
    from concourse import bacc, mybir

    f32 = mybir.dt.float32
    f8 = mybir.dt.float8e4

    nc = bacc.Bacc("TRN2", target_bir_lowering=False, debug=False,
                   num_devices=NCORES)
    hta = nc.declare_dram_parameter("hta", [NGRAN, P, 2, HCHK, PEBLK * BCOL],
                                    f8, isOutput=False)
    htb = nc.declare_dram_parameter("htb", [NGRAN, P, 2, GRAN // 2], f8,
                                    isOutput=False)
    stats = nc.declare_dram_parameter("stats", [NGRAN, P, PEBLK, BCOL], f32,
                                      isOutput=True)
    # block-3 outputs packed in one tensor: cols 0..7 = half-granule sumsq,
    # cols 8..71 = chunk sums
    blk3 = nc.declare_dram_parameter("blk3", [P, NHALF + NGRAN * NCHK], f32,
                                     isOutput=True)

    with tile.TileContext(nc) as tc:
        with (
            tc.tile_pool(name="in", bufs=1) as in_pool,
            tc.tile_pool(name="st", bufs=2) as stage_pool,
            tc.tile_pool(name="sc", bufs=2) as scr_pool,
            tc.tile_pool(name="acc", bufs=1) as acc_pool,
            tc.tile_pool(name="ps", bufs=2, space="PSUM") as psum_pool,
        ):
            b3_t = acc_pool.tile([P, NHALF + NGRAN * NCHK], f32, tag="b3")
            sq3_t = b3_t[:, :NHALF]
            sm3_t = b3_t[:, NHALF:]

            # input DMAs up front, in consumption order, minimizing trigger
            # count (each trigger costs ~610ns of serial SP time plus a DGE
            # ramp): granule 0's PE data split in halves so matmuls start
            # earliest.
            atiles = {}
            btiles = []
            for g in range(NGRAN):
                if g == 0:
                    for h in range(2):
                        t = in_pool.tile([P, 1, HCHK, PEBLK * BCOL], f8,
                                         tag=f"t0{h}")
                        nc.sync.dma_start(t[:, 0], hta[0][:, h])
                        atiles[(0, h)] = t[:, 0]
                else:
                    t = in_pool.tile([P, 2, HCHK, PEBLK * BCOL], f8,
                                     tag=f"t{g}")
                    nc.sync.dma_start(t[:], hta[g])
                    atiles[(g, 0)] = t[:, 0]
                    atiles[(g, 1)] = t[:, 1]
                tb = in_pool.tile([P, 2, GRAN // 2], f8, tag=f"b{g}")
                nc.sync.dma_start(tb[:], htb[g])
                btiles.append(tb)

            for g in range(NGRAN):
                # block 3 on ACT (sumsq) + DVE (sums), per half-granule
                for h in range(2):
                    tb = btiles[g][:][:, h]              # [128, 1024] fp8
                    scr = scr_pool.tile([P, GRAN // 2], f32, tag="scr")
                    nc.scalar.activation(
                        scr[:], tb, mybir.ActivationFunctionType.Square,
                        accum_out=sq3_t[:, 2 * g + h:2 * g + h + 1])
                    tb3 = tb.rearrange("p (c x) -> p c x", x=P)
                    nc.vector.reduce_sum(
                        sm3_t[:, g * NCHK + h * HCHK:g * NCHK + (h + 1) * HCHK],
                        tb3, axis=mybir.AxisListType.X)

                # blocks 0-2 on the TensorEngine
                pt = psum_pool.tile([P, PEBLK, 512], f32, tag="ps")
                stage = stage_pool.tile([P, PEBLK, BCOL], f32, tag="st")
                for b in range(PEBLK):
                    for ch in range(NCHK):
                        th = atiles[(g, ch // HCHK)]
                        lc = ch % HCHK
                        stat_ap = th[:, lc, b * BCOL:b * BCOL + P]
                        mov_ap = th[:, lc, b * BCOL:b * BCOL + BCOL]
                        nc.tensor.matmul(
                            pt[:, b, 0:BCOL], stat_ap, mov_ap,
                            start=(ch == 0), stop=(ch == NCHK - 1))
                nc.scalar.copy(stage[:], pt[:, :, 0:BCOL])
                nc.sync.dma_start(stats[g], stage[:])

            nc.sync.dma_start(blk3[:], b3_t[:])
    nc.compile()
    return nc


def _get_nc():
    if "nc" not in _NC_CACHE:
        _NC_CACHE["nc"] = _build_nc()
    return _NC_CACHE["nc"]


def _granule_classes(ids_sorted, size):
    """Per-granule class id, or -1 if the granule spans a class boundary."""
    g = ids_sorted.reshape(-1, size)
    pure = g[:, 0] == g[:, -1]
    return np.where(pure, g[:, 0], -1).astype(np.int64)


def _prep_core(hs_k, ids_k):
    """hs_k/ids_k already globally sorted. Returns device input + host fixups."""
    q = hs_k.astype(F8)
    q5 = q[:, :PEBLK * P].reshape(NGRAN, NCHK, P, PEBLK, P)
    buf = np.empty((NGRAN, P, NCHK, PEBLK, BCOL), dtype=F8)
    buf[..., :P] = q5.transpose(0, 2, 1, 3, 4)
    buf[..., P] = np.array(1.0, dtype=F8)
    hta = buf.reshape(NGRAN, P, 2, HCHK, PEBLK * BCOL)
    htb = np.ascontiguousarray(
        q[:, PEBLK * P:].reshape(NGRAN, 2, GRAN // 2, P).transpose(0, 3, 1, 2))

    gcls = _granule_classes(ids_k, GRAN)          # [4]

    bsum = np.zeros((C, D), dtype=np.float64)
    bsq = np.zeros((C, D), dtype=np.float64)
    # transition granules: host computes their per-class stats exactly
    if (gcls < 0).any():
        m = np.repeat(gcls < 0, GRAN)
        rows, rids = hs_k[m].astype(np.float64), ids_k[m]
        for q in np.unique(rids):
            sel = rows[rids == q]
            bsum[q] += sel.sum(axis=0)
            bsq[q] += (sel * sel).sum(axis=0)
    return hta, htb, gcls, bsum, bsq


def _device_stats(hidden, ids, **run_kwargs):
    """Returns (sums[C,D], sumsq[C,D]) float64, plus the raw run result."""
    from concourse import bass_utils

    nc = _get_nc()

    order = np.argsort(ids, kind="stable")       # GLOBAL sort by class
    ids_s = ids[order]
    hs = hidden[order]

    in_maps = []
    meta = []
    sums = np.zeros((C, D), dtype=np.float64)
    sumsq = np.zeros((C, D), dtype=np.float64)
    for k in range(NCORES):
        rows = slice(k * ROWS, (k + 1) * ROWS)
        hta, htb, gcls, bsum, bsq = _prep_core(hs[rows], ids_s[rows])
        in_maps.append({"hta": hta, "htb": htb})
        meta.append(gcls)
        sums += bsum
        sumsq += bsq

    res = bass_utils.run_bass_kernel_spmd(nc, in_maps, list(range(NCORES)),
                                          **run_kwargs)

    DPE = PEBLK * P  # 384 features on the PE path
    for k in range(NCORES):
        gcls = meta[k]
        st = res.results[k]["stats"].astype(np.float64)  # [4, 128, 3, 129]
        # [g, f, b] -> [g, b, f] -> [g, 384] (feature id = b*128 + f)
        gsums = np.empty((NGRAN, D))
        gsq = np.empty((NGRAN, D))
        gsums[:, :DPE] = st[:, :, :, P].transpose(0, 2, 1).reshape(NGRAN, DPE)
        gsq[:, :DPE] = np.diagonal(
            st[:, :, :, :P], axis1=1, axis2=3).reshape(NGRAN, DPE)
        b3 = res.results[k]["blk3"].astype(np.float64)   # [128, 72]
        sq3 = b3[:, :NHALF]                              # [128, 8]
        sm3 = b3[:, NHALF:]                              # [128, 64]
        gsq[:, DPE:] = (sq3[:, 0::2] + sq3[:, 1::2]).T
        gsums[:, DPE:] = sm3.reshape(P, NGRAN, NCHK).sum(axis=2).T
        for g in range(NGRAN):
            c = gcls[g]
            if c >= 0:
                sums[c] += gsums[g]
                sumsq[c] += gsq[g]
    return sums, sumsq, res


def _pairwise_loss(counts, sums, sumsq, d):
    """The tiny O(C^2 D) stage on host CPU.

    Runs in float32 with the same jax ops as the reference: at these extreme
    betainc parameters (b ~ 8190, x ~ 1e-5) jax's f32 betainc differs from
    the true (f64) value by ~1e-3, so matching the reference requires
    replicating its f32 numerics, not improving on them.
    """
    import jax
    import jax.numpy as jnp

    cpu = jax.devices("cpu")[0]
    with jax.default_device(cpu):
        counts64 = counts.astype(np.float64)
        means64 = sums / counts64[:, None]
        withins64 = sumsq - counts64[:, None] * means64**2
        counts = jnp.asarray(counts64, jnp.float32)               # [C]
        means = jnp.asarray(means64, jnp.float32)                 # [C, D]
        withins = jnp.asarray(withins64, jnp.float32)             # [C, D]
        half_diff = (means[:, None, :] - means[None, :, :]) * 0.5
        pair_counts = counts[:, None] + counts[None, :]
        pair_between = half_diff * half_diff * pair_counts[:, :, None]
        pair_within = withins[:, None, :] + withins[None, :, :]
        d2 = pair_counts - 2.0
        d2 = jnp.where(d2 == 0.0, 1e-5, d2)
        x = pair_between / (pair_between + pair_within)
        x = jnp.clip(x, XMIN, XMAX)
        a = jnp.full_like(x, 0.5)
        b = jnp.broadcast_to((d2 * 0.5)[:, :, None], x.shape)
        xbetainc = jax.scipy.special.betainc(a, b, x)             # [C, C, D]
        top_k, _ = jax.lax.top_k(xbetainc, int(d))                # [C, C, d]
        per_pair = jnp.sum(jnp.log(top_k), axis=-1)               # [C, C]
        mask = jnp.triu(jnp.ones((C, C), dtype=bool), k=1)
        total = jnp.sum(jnp.where(mask, per_pair, jnp.zeros_like(per_pair)))
        return float(-total)


def kernel(hidden, batch_ids, d):
    hidden = np.asarray(hidden, dtype=np.float32)
    ids = np.asarray(batch_ids).astype(np.int64)
    assert hidden.shape == (N, D), hidden.shape

    counts = np.bincount(ids, minlength=C).astype(np.float64)
    sums, sumsq, _ = _device_stats(hidden, ids)
    total = _pairwise_loss(counts, sums, sumsq, int(np.asarray(d)))
    return np.array(total, dtype=np.float32)



# revision 10
# speedup vs baseline: 1.0899x; 1.0899x over previous
"""Bass/Trainium2 kernel for nn_F_Loss_65446711656630.

Strategy (data-parallel over N, 8 cores):
  - Host: GLOBAL stable sort of all rows by class id, quantize to fp8 e4m3
    (final loss rel err ~1.4e-4, well under tolerance), then lay out
    per-core operands split across engines:
      * hta  (features 0-255, all granules + features 256-383 of granule 3):
        row-major for the PE Gram path.
      * htb  (features 256-383 g0-g2, features 384-511 all granules):
        feature-major for DVE/ACT.
  - Device (static kernel), work split across engines (measured rates:
    MATMUL N/2.4 ns + LDWEIGHTS P/1.2 ns but ~2x slower while DMA is
    saturating HBM due to the ~50% utilization power throttle; ACT
    (N+352)/1.2 ns; DVE ~1.19 ns/elem):
      * TensorE (18/32 units): per 128-row chunk, fp8 matmul with
        stationary = X_b, moving = [X_b | 1] accumulating X_b^T X_b
        (diag = sumsq) and X_b^T 1 (sums) into PSUM; stats DMAed
        directly from PSUM (no stage copy).
      * DVE (10/32 units): bn_stats (count/mean/count*var per 512-elem
        subtile) gives BOTH sum and sumsq in a single pass.
      * ACT (4/32 units): Square+accum_out pass (sumsq) and
        Copy+accum_out pass (sums) per granule.
      * DMA triggers split between Sync and ACT queues (each costs
        ~650ns of serial sequencer time).
  - Host: per-class stats from single-class granule partials (f64) +
    direct numpy f64 sums for class-transition granules; then the tiny
    O(C^2 D) pairwise betainc/top-k stage in f32 jax on CPU (mirroring
    the reference's numerics exactly).
"""

import os

import ml_dtypes
import numpy as np

# safety net: recover cleanly if a previous process left a NeuronCore wedged
os.environ.setdefault("NEURON_RT_RESET_CORES", "1")

C = 16
D = 512
N = 65536
NCORES = 8
ROWS = N // NCORES          # 8192 rows per core
P = 128                     # SBUF partitions
GRAN = 2048                 # rows per granule (stats accumulation unit)
NGRAN = ROWS // GRAN        # 4 granules per core
NCHK = GRAN // P            # 16 chunks per granule
HCHK = NCHK // 2            # 8 chunks per half-granule
BCOL = P + 1                # 129 cols per PE block: 128 features + ones col
XMIN, XMAX = 1e-37, 1.0 - 1e-5

# engine split of the (granule, feature-block) grid:
#   features   0-255 (blocks 0,1): PE, all granules           (16 units)
#   features 256-383 (block 2):    DVE g0-g2, PE g3           (6+2 units)
#   features 384-511 (block 3):    ACT g0-g1, DVE g2-g3       (4+4 units)
ACC_COLS = 124              # DVE bn stats (120) + ACT accums (4)

F8 = ml_dtypes.float8_e4m3

_NC_CACHE = {}


def _build_nc():
    """Per-core SPMD program.

    Inputs:  "hta"  [4, 128, 2, 8, 258] fp8e4  (granule, partition, half,
               chunk, 2 blocks x [128 feat | 1.0]; row r within granule =
               (half*8 + chunk)*128 + p)
             "hta2" [128, 2, 8, 129] fp8e4  (granule 3, block 2, row-major)
             "htb"  [128, 14, 1024] fp8e4  (feature-major halves, order:
               g0:[b2h0,b2h1,b3h0,b3h1] g1:[...] g2:[...] g3:[b3h0,b3h1])
    Outputs: "pe"   [4, 128, 2, 129] f32  (Gram blocks: pe[g,f,b,c] =
               sum over granule g of X[:,b*128+f]*X[:,b*128+c] for c<128,
               plain sum of X[:,b*128+f] at c==128)
             "pe2"  [128, 129] f32  (same for granule 3, block 2)
             "acc"  [128, 124] f32  (cols 24g..24g+23: bn_stats of block 2
               granule g (g<3) as [4 subtiles x 6]; cols 72..95: bn block 3
               granule 2; cols 96..119: bn block 3 granule 3; cols 120-123:
               ACT sumsq/sum for block 3 granules 0,1)
    """
    import concourse.tile as tile
    from concourse import bacc, mybir

    f32 = mybir.dt.float32
    bf16 = mybir.dt.bfloat16
    f8 = mybir.dt.float8e4

    nc = bacc.Bacc("TRN2", target_bir_lowering=False, debug=False,
                   num_devices=NCORES)
    hta = nc.declare_dram_parameter("hta", [NGRAN, P, 2, HCHK, 2 * BCOL],
                                    f8, isOutput=False)
    hta2 = nc.declare_dram_parameter("hta2", [P, 2, HCHK, BCOL], f8,
                                     isOutput=False)
    htb = nc.declare_dram_parameter("htb", [P, 14, 1024], f8, isOutput=False)
    pe = nc.declare_dram_parameter("pe", [NGRAN, P, 2, BCOL], f32,
                                   isOutput=True)
    pe2 = nc.declare_dram_parameter("pe2", [P, BCOL], f32, isOutput=True)
    accp = nc.declare_dram_parameter("acc", [P, ACC_COLS], f32, isOutput=True)

    with tile.TileContext(nc) as tc:
        with (
            tc.tile_pool(name="in", bufs=1) as in_pool,
            tc.tile_pool(name="sc", bufs=2) as scr_pool,
            tc.tile_pool(name="st", bufs=2) as stg_pool,
            tc.tile_pool(name="acc", bufs=1) as acc_pool,
            tc.tile_pool(name="ps", bufs=2, space="PSUM") as psum_pool,
            tc.tile_pool(name="ps2", bufs=1, space="PSUM") as psum2_pool,
        ):
            acc_t = acc_pool.tile([P, ACC_COLS], f32, tag="acc")

            # ---- input DMAs, split across the two HWDGE queues --------
            # Sync: PE-path data + g0 htb (so DVE/ACT start early); the
            # first transfer is half a granule so matmuls start ASAP.
            ta = []
            tb = []
            t0 = in_pool.tile([P, 2, HCHK, 2 * BCOL], f8, tag="ta0")
            nc.sync.dma_start(t0[:, 0], hta[0][:, 0])
            ta.append(t0)
            tb0 = in_pool.tile([P, 4, 1024], f8, tag="tb0", name="tb0")
            nc.sync.dma_start(tb0[:], htb[:, 0:4])
            tb.append(tb0)
            nc.sync.dma_start(t0[:, 1], hta[0][:, 1])
            t1 = in_pool.tile([P, 2, HCHK, 2 * BCOL], f8, tag="ta1")
            nc.sync.dma_start(t1[:], hta[1])
            ta.append(t1)
            tx = in_pool.tile([P, 2, HCHK, BCOL], f8, tag="tx")
            nc.sync.dma_start(tx[:], hta2[:])
            for g in range(2, NGRAN):
                t = in_pool.tile([P, 2, HCHK, 2 * BCOL], f8, tag=f"ta{g}",
                                 name=f"ta{g}")
                nc.sync.dma_start(t[:], hta[g])
                ta.append(t)
            # ACT: remaining feature-major data
            for g in range(1, NGRAN):
                nh = 4 if g < 3 else 2
                t = in_pool.tile([P, nh, 1024], f8, tag=f"tb{g}",
                                 name=f"tb{g}")
                nc.scalar.dma_start(t[:], htb[:, 4 * g:4 * g + nh])
                tb.append(t)

            # ---- TensorE: Gram blocks 0,1 all granules + block 2 g3 ---
            # (chunk-major so half a granule of data is enough to start;
            # the hta2 block runs mid-stream so its stage copy isn't the
            # critical-path tail)
            stg = []

            def gram(g):
                pt = psum_pool.tile([P, 2, 512], f32, tag="ps", name="pt")
                for ch in range(NCHK):
                    th = ta[g][:, ch // HCHK, ch % HCHK]
                    for b in range(2):
                        nc.tensor.matmul(
                            pt[:, b, 0:BCOL],
                            th[:, b * BCOL:b * BCOL + P],
                            th[:, b * BCOL:b * BCOL + BCOL],
                            start=(ch == 0), stop=(ch == NCHK - 1))
                return pt

            pts = [gram(0), gram(1)]
            pt2 = psum2_pool.tile([P, 512], f32, tag="ps2")
            for u in range(2):
                for ch in range(HCHK):
                    th = tx[:, u, ch]
                    nc.tensor.matmul(
                        pt2[:, 0:BCOL], th[:, 0:P], th[:, 0:BCOL],
                        start=(u == 0 and ch == 0),
                        stop=(u == 1 and ch == HCHK - 1))
            pts += [gram(2), gram(3)]

            # ---- DVE: bn_stats (sum+sumsq in one pass) + late copies --
            # (hardware caps BN_STATS at 512 elements per instruction)
            def bn(dst_col, src_ap, nsub):
                flat = src_ap.rearrange("p a (b x) -> p (a b) x", x=512)
                for i in range(nsub):
                    nc.vector.bn_stats(
                        acc_t[:, dst_col + 6 * i:dst_col + 6 * i + 6],
                        flat[:, i])

            for g in range(3):                       # block 2, g0-g2
                bn(24 * g, tb[g][:, 0:2, :], 4)
            bn(72, tb[2][:, 2:4, :], 4)              # block 3, g2
            bn(96, tb[3][:, 0:2, :], 4)              # block 3, g3
            for g in (2, 3):
                s = stg_pool.tile([P, 2, BCOL], f32, tag=f"sg{g}",
                                  name=f"sg{g}")
                nc.vector.tensor_copy(s[:], pts[g][:, :, 0:BCOL])
                stg.append((g, s))

            # ---- ACT: block 3 granules 0,1 + early stage copies -------
            scr = scr_pool.tile([P, 2, 1024], bf16, tag="scr")
            nc.scalar.activation(
                scr[:], tb[0][:, 2:4, :], mybir.ActivationFunctionType.Square,
                accum_out=acc_t[:, 120:121])
            scr2 = scr_pool.tile([P, 2, 1024], bf16, tag="scr2")
            nc.scalar.activation(
                scr2[:], tb[0][:, 2:4, :], mybir.ActivationFunctionType.Copy,
                accum_out=acc_t[:, 121:122])
            s0 = stg_pool.tile([P, 2, BCOL], f32, tag="sg0")
            nc.scalar.copy(s0[:], pts[0][:, :, 0:BCOL])
            scr3 = scr_pool.tile([P, 2, 1024], bf16, tag="scr3")
            nc.scalar.activation(
                scr3[:], tb[1][:, 2:4, :], mybir.ActivationFunctionType.Square,
                accum_out=acc_t[:, 122:123])
            scr4 = scr_pool.tile([P, 2, 1024], bf16, tag="scr4")
            nc.scalar.activation(
                scr4[:], tb[1][:, 2:4, :], mybir.ActivationFunctionType.Copy,
                accum_out=acc_t[:, 123:124])
            s1 = stg_pool.tile([P, 2, BCOL], f32, tag="sg1")
            nc.scalar.copy(s1[:], pts[1][:, :, 0:BCOL])
            s2x = stg_pool.tile([P, BCOL], f32, tag="sg2x")
            nc.scalar.copy(s2x[:], pt2[:, 0:BCOL])

            # ---- output DMAs (Sync queue; ACT sends the final acc) ----
            nc.sync.dma_start(pe[0], s0[:])
            nc.sync.dma_start(pe[1], s1[:])
            nc.sync.dma_start(pe2[:], s2x[:])
            for g, s in stg:
                nc.sync.dma_start(pe[g], s[:])
            nc.scalar.dma_start(accp[:], acc_t[:])
    nc.compile()
    return nc


def _get_nc():
    if "nc" not in _NC_CACHE:
        _NC_CACHE["nc"] = _build_nc()
    return _NC_CACHE["nc"]


def _granule_classes(ids_sorted, size):
    """Per-granule class id, or -1 if the granule spans a class boundary."""
    g = ids_sorted.reshape(-1, size)
    pure = g[:, 0] == g[:, -1]
    return np.where(pure, g[:, 0], -1).astype(np.int64)


def _prep_core(hs_k, ids_k):
    """hs_k/ids_k already globally sorted. Returns device inputs + host fixups."""
    q = hs_k.astype(F8)

    # hta: features 0-255, row-major with interleaved ones columns
    q5 = q[:, :2 * P].reshape(NGRAN, NCHK, P, 2, P)
    buf = np.empty((NGRAN, P, NCHK, 2, BCOL), dtype=F8)
    buf[..., :P] = q5.transpose(0, 2, 1, 3, 4)
    buf[..., P] = np.array(1.0, dtype=F8)
    hta = buf.reshape(NGRAN, P, 2, HCHK, 2 * BCOL)

    # hta2: granule 3, features 256-383, row-major
    q2 = q[3 * GRAN:, 2 * P:3 * P].reshape(NCHK, P, P)
    buf2 = np.empty((P, NCHK, BCOL), dtype=F8)
    buf2[..., :P] = q2.transpose(1, 0, 2)
    buf2[..., P] = np.array(1.0, dtype=F8)
    hta2 = buf2.reshape(P, 2, HCHK, BCOL)

    # htb: feature-major halves
    htb = np.empty((P, 14, 1024), dtype=F8)
    col = 0
    for g in range(NGRAN):
        blocks = (2, 3) if g < 3 else (3,)
        for b in blocks:
            seg = q[g * GRAN:(g + 1) * GRAN, b * P:(b + 1) * P]  # [2048,128]
            htb[:, col:col + 2] = seg.T.reshape(P, 2, 1024)
            col += 2
    assert col == 14

    gcls = _granule_classes(ids_k, GRAN)          # [4]

    bsum = np.zeros((C, D), dtype=np.float64)
    bsq = np.zeros((C, D), dtype=np.float64)
    # transition granules: host computes their per-class stats exactly
    if (gcls < 0).any():
        m = np.repeat(gcls < 0, GRAN)
        rows, rids = hs_k[m].astype(np.float64), ids_k[m]
        for cq in np.unique(rids):
            sel = rows[rids == cq]
            bsum[cq] += sel.sum(axis=0)
            bsq[cq] += (sel * sel).sum(axis=0)
    return {"hta": hta, "hta2": hta2, "htb": htb}, gcls, bsum, bsq


def _decode_bn(block):
    """block: [128, 4, 6] f64 -> (sums[128], sumsq[128])."""
    ce, me, ve = block[:, :, 0], block[:, :, 1], block[:, :, 2]
    co, mo, vo = block[:, :, 3], block[:, :, 4], block[:, :, 5]
    sums = (ce * me + co * mo).sum(axis=1)
    sumsq = (ve + ce * me * me + vo + co * mo * mo).sum(axis=1)
    return sums, sumsq


def _device_stats(hidden, ids, **run_kwargs):
    """Returns (sums[C,D], sumsq[C,D]) float64, plus the raw run result."""
    from concourse import bass_utils

    nc = _get_nc()

    order = np.argsort(ids, kind="stable")       # GLOBAL sort by class
    ids_s = ids[order]
    hs = hidden[order]

    in_maps = []
    meta = []
    sums = np.zeros((C, D), dtype=np.float64)
    sumsq = np.zeros((C, D), dtype=np.float64)
    for k in range(NCORES):
        rows = slice(k * ROWS, (k + 1) * ROWS)
        im, gcls, bsum, bsq = _prep_core(hs[rows], ids_s[rows])
        in_maps.append(im)
        meta.append(gcls)
        sums += bsum
        sumsq += bsq

    res = bass_utils.run_bass_kernel_spmd(nc, in_maps, list(range(NCORES)),
                                          **run_kwargs)

    for k in range(NCORES):
        gcls = meta[k]
        st = res.results[k]["pe"].astype(np.float64)     # [4, 128, 2, 129]
        st2 = res.results[k]["pe2"].astype(np.float64)   # [128, 129]
        acc = res.results[k]["acc"].astype(np.float64)   # [128, 124]

        gsums = np.empty((NGRAN, D))
        gsq = np.empty((NGRAN, D))
        # features 0-255 from PE Gram blocks (feature = b*128 + f)
        gsums[:, :2 * P] = st[:, :, :, P].transpose(0, 2, 1).reshape(NGRAN,
                                                                     2 * P)
        gsq[:, :2 * P] = np.diagonal(st[:, :, :, :P], axis1=1,
                                     axis2=3).reshape(NGRAN, 2 * P)
        # features 256-383: DVE bn for g0-g2, PE Gram for g3
        for g in range(3):
            s, sq = _decode_bn(acc[:, 24 * g:24 * g + 24].reshape(P, 4, 6))
            gsums[g, 2 * P:3 * P] = s
            gsq[g, 2 * P:3 * P] = sq
        gsums[3, 2 * P:3 * P] = st2[:, P]
        gsq[3, 2 * P:3 * P] = np.diagonal(st2[:, :P])
        # features 384-511: ACT accums for g0-g1, DVE bn for g2-g3
        for g in range(2):
            gsq[g, 3 * P:] = acc[:, 120 + 2 * g]
            gsums[g, 3 * P:] = acc[:, 121 + 2 * g]
        s, sq = _decode_bn(acc[:, 72:96].reshape(P, 4, 6))
        gsums[2, 3 * P:] = s
        gsq[2, 3 * P:] = sq
        s, sq = _decode_bn(acc[:, 96:120].reshape(P, 4, 6))
        gsums[3, 3 * P:] = s
        gsq[3, 3 * P:] = sq

        for g in range(NGRAN):
            c = gcls[g]
            if c >= 0:
                sums[c] += gsums[g]
                sumsq[c] += gsq[g]
    return sums, sumsq, res


def _pairwise_loss(counts, sums, sumsq, d):
    """The tiny O(C^2 D) stage on host CPU.

    Runs in float32 with the same jax ops as the reference: at these extreme
    betainc parameters (b ~ 8190, x ~ 1e-5) jax's f32 betainc differs from
    the true (f64) value by ~1e-3, so matching the reference requires
    replicating its f32 numerics, not improving on them.
    """
    import jax
    import jax.numpy as jnp

    cpu = jax.devices("cpu")[0]
    with jax.default_device(cpu):
        counts64 = counts.astype(np.float64)
        means64 = sums / counts64[:, None]
        withins64 = sumsq - counts64[:, None] * means64**2
        counts = jnp.asarray(counts64, jnp.float32)               # [C]
        means = jnp.asarray(means64, jnp.float32)                 # [C, D]
        withins = jnp.asarray(withins64, jnp.float32)             # [C, D]
        half_diff = (means[:, None, :] - means[None, :, :]) * 0.5
        pair_counts = counts[:, None] + counts[None, :]
        pair_between = half_diff * half_diff * pair_counts[:, :, None]
        pair_within = withins[:, None, :] + withins[None, :, :]
        d2 = pair_counts - 2.0
        d2 = jnp.where(d2 == 0.0, 1e-5, d2)
        x = pair_between / (pair_between + pair_within)
        x = jnp.clip(x, XMIN, XMAX)
        a = jnp.full_like(x, 0.5)
        b = jnp.broadcast_to((d2 * 0.5)[:, :, None], x.shape)
        xbetainc = jax.scipy.special.betainc(a, b, x)             # [C, C, D]
        top_k, _ = jax.lax.top_k(xbetainc, int(d))                # [C, C, d]
        per_pair = jnp.sum(jnp.log(top_k), axis=-1)               # [C, C]
        mask = jnp.triu(jnp.ones((C, C), dtype=bool), k=1)
        total = jnp.sum(jnp.where(mask, per_pair, jnp.zeros_like(per_pair)))
        return float(-total)


def kernel(hidden, batch_ids, d):
    hidden = np.asarray(hidden, dtype=np.float32)
    ids = np.asarray(batch_ids).astype(np.int64)
    assert hidden.shape == (N, D), hidden.shape

    counts = np.bincount(ids, minlength=C).astype(np.float64)
    sums, sumsq, _ = _device_stats(hidden, ids)
    total = _pairwise_loss(counts, sums, sumsq, int(np.asarray(d)))
    return np.array(total, dtype=np.float32)
